# revision 12
# baseline (speedup 1.0000x reference)
"""Multi-head attention (B=4, S=2048, D=1024, H=16) on 8 Trainium2 NeuronCores.

Sharding: core c handles batch b = c//2 and query-row half c%2 (1024 query
rows). Each core computes K/V projections for its batch's full 2048 kv rows
(duplicated across the pair), attention for its 1024 query rows over all 16
heads, and the output projection for its rows. Output is a pure concatenation
across cores - no collectives.

v2 schedule, rebuilt around the measured bottlenecks of the 494us baseline:
  - scores K=64 parity pairs issue back-to-back on disjoint PE row groups
    (partitions 0-63 / 64-127) and run CONCURRENTLY (HW-verified 124ns/MM
    vs 428 serialized). Both parities of a kv-block write one [128,2,512]
    PSUM tile, so the pair's buffer frees as a unit at window start and the
    pair never splits on a semaphore.
  - one PSUM ring (tag "s", 3 x 4KB = 6 banks) serves scores AND every
    projection; rides allocate in pairs to keep the ring's free-order
    aligned with window boundaries. attnV accumulators keep 2 banks.
  - all projections (Q/K/V/out) ride inside the exp-paced windows via a
    deadline-sorted queue (forced pop 2 windows before need).
  - et-sliced Wq/Wk layouts let the first matmul start after ~300KB of DMA;
    kt/wq/wk live in small et-rings; yt shares storage with qt (the qt
    slice is dead exactly when normalize writes yt); output is bf16.

Per-core compute (all matmuls bf16 with fp32 PSUM accumulation):
  QT[e,s]  = (Wq xq)/8 + bq/8
  KT[e,s]  =  Wk xk                 (bk dropped: softmax-invariant)
  V[s,e]   =  xv Wv^T               (bv folded into output bias)
  per head-pair hp = et, q-chunk qc, window k (kv blocks 2k, 2k+1):
    scoresT[kv,q] = KT_h^T @ QT_h   (K=64 row-tiled concurrent parity pair)
    E = exp(scoresT)                (ScalarE, PSUM->SBUF bf16 - the pacer)
    [y_h; rowsum] += [V_h | 1]^T @ E    (M=65, ones col gives rowsum)
    yT_h = y_h * (1/rowsum)
  out[q,e] = yT^T Wo + (bo + Wo bv), stored bf16 (host casts to f32)
"""

import numpy as np
import ml_dtypes

B, S, D = 4, 2048, 1024
H, DK = 16, 64
NCORES = 8
SQ = S // 2            # query rows per core
P = 128
ET = D // P            # 8 output-dim tiles (= head pairs)
DT = D // P            # 8 contraction tiles
KT_N = S // P          # 16 kv blocks
HP = H // 2            # 8 head pairs
CHUNK = 512
WIN = KT_N // 2        # 8 windows per stream
KTR = 3                # kt et-ring depth
WR = 4                 # wq/wk et-ring depth

STREAMS = [(hp, qc) for hp in range(HP) for qc in (0, 1)]

_CACHE = {}


def _build_nc():
    import concourse.mybir as mybir
    import concourse.tile as tile
    from concourse import bacc

    F32, BF16 = mybir.dt.float32, mybir.dt.bfloat16
    Exp = mybir.ActivationFunctionType.Exp

    nc = bacc.Bacc("TRN2", target_bir_lowering=False, debug=False,
                   num_devices=NCORES)

    xqT = nc.dram_tensor("xqT", [D, SQ], BF16, kind="ExternalInput").ap()
    xkT = nc.dram_tensor("xkT", [D, S], BF16, kind="ExternalInput").ap()
    xvT = nc.dram_tensor("xvT", [D, S], BF16, kind="ExternalInput").ap()
    # et-sliced: [p, et, dt, m] = W[et*128+m, dt*128+p]
    wqE = nc.dram_tensor("wqE", [P, ET * DT * P], BF16,
                         kind="ExternalInput").ap()
    wkE = nc.dram_tensor("wkE", [P, ET * DT * P], BF16,
                         kind="ExternalInput").ap()
    wvT = nc.dram_tensor("wvT", [D, D], BF16, kind="ExternalInput").ap()
    woT = nc.dram_tensor("woT", [D, D], BF16, kind="ExternalInput").ap()
    bqs = nc.dram_tensor("bqs", [P, ET], F32, kind="ExternalInput").ap()
    bob = nc.dram_tensor("bob", [P, D], BF16, kind="ExternalInput").ap()
    out = nc.dram_tensor("out", [SQ, D], BF16, kind="ExternalOutput").ap()

    wqV = wqE.rearrange("p (e d m) -> p e d m", e=ET, d=DT)
    wkV = wkE.rearrange("p (e d m) -> p e d m", e=ET, d=DT)

    def pdt(ap):  # [D, N] dram -> [P, DT, N] partition-tiled view
        return ap.rearrange("(a p) n -> p a n", p=P)

    with tile.TileContext(nc) as tc:
        with (
            tc.tile_pool(name="cpool", bufs=1) as cpool,
            tc.tile_pool(name="xkp", bufs=2) as xkp,
            tc.tile_pool(name="xvp", bufs=2) as xvp,
            tc.tile_pool(name="epool", bufs=16) as epool,
            tc.tile_pool(name="npool", bufs=2) as npool,
            tc.tile_pool(name="opool", bufs=2) as opool,
            tc.tile_pool(name="psS", bufs=3, space="PSUM") as psS,
            tc.tile_pool(name="psA", bufs=2, space="PSUM") as psA,
        ):
            # ---- residents ----
            bq_s = cpool.tile([P, ET], F32, name="bq_s")
            nc.sync.dma_start(bq_s[:], bqs[:])
            bob_s = cpool.tile([P, D], BF16, name="bob_s")
            nc.sync.dma_start(bob_s[:], bob[:])
            ones_s = cpool.tile([1, DK], BF16, name="ones_s")
            nc.gpsimd.memset(ones_s[:], 1.0)

            # qt for (et,qc) until its stream's normalize overwrites it
            # with yt; outproj reads it as yt.
            qy_s = cpool.tile([P, ET, SQ], BF16, name="qy_s")
            kt_s = cpool.tile([P, KTR, S], BF16, name="kt_s")
            wq_s = cpool.tile([P, WR, DT, P], BF16, name="wq_s")
            wk_s = cpool.tile([P, WR, DT, P], BF16, name="wk_s")
            wv_s = cpool.tile([P, DT, D], BF16, name="wv_s")
            wo_s = cpool.tile([P, DT, D], BF16, name="wo_s")
            xq_s = cpool.tile([P, 2, DT, CHUNK], BF16, name="xq_s")
            va_s = cpool.tile([P, KT_N, H * (DK + 1)], BF16, name="va_s")
            va_ones = va_s.rearrange("p k (h c) -> p k h c", c=DK + 1)
            nc.gpsimd.memset(va_ones[:, :, :, DK:DK + 1], 1.0)

            # ---- DMA helpers ----
            def dma_wslice(wsb, wview, et, eng=None):
                (eng or nc.gpsimd).dma_start(wsb[:, et % WR, :, :],
                                             wview[:, et, :, :])

            def dma_xq(qc):
                nc.sync.dma_start(
                    xq_s[:, qc, :, :],
                    pdt(xqT[:, qc * CHUNK:(qc + 1) * CHUNK]))

            xk_tiles = {}

            def dma_xk(et, sc):
                xc = xkp.tile([P, DT, CHUNK], BF16, tag="xk", name="xk_c")
                nc.sync.dma_start(
                    xc[:], pdt(xkT[:, sc * CHUNK:(sc + 1) * CHUNK]))
                xk_tiles[et, sc] = xc

            xv_tiles = {}

            def dma_xv(c):
                xc = xvp.tile([P, DT, CHUNK], BF16, tag="xv", name="xv_c")
                nc.sync.dma_start(
                    xc[:], pdt(xvT[:, c * CHUNK:(c + 1) * CHUNK]))
                xv_tiles[c] = xc

            # ---- compute blocks (each allocates exactly one "s" tile) ----
            def qproj(et, qc):
                psq = psS.tile([P, CHUNK], F32, tag="s", name="psq")
                for dt in range(DT):
                    nc.tensor.matmul(psq[:], wq_s[:, et % WR, dt, :],
                                     xq_s[:, qc, dt, :],
                                     start=(dt == 0), stop=(dt == DT - 1))
                nc.vector.tensor_scalar(
                    qy_s[:, et, qc * CHUNK:(qc + 1) * CHUNK], psq[:],
                    0.125, bq_s[:, et:et + 1],
                    mybir.AluOpType.mult, mybir.AluOpType.add)

            def kproj(et, sc):
                xc = xk_tiles.pop((et, sc))
                psk = psS.tile([P, CHUNK], F32, tag="s", name="psk")
                for dt in range(DT):
                    nc.tensor.matmul(psk[:], wk_s[:, et % WR, dt, :],
                                     xc[:, dt, :],
                                     start=(dt == 0), stop=(dt == DT - 1))
                nc.vector.tensor_copy(
                    kt_s[:, et % KTR, sc * CHUNK:(sc + 1) * CHUNK], psk[:])

            def vproj_half(st, ec):
                xc = xv_tiles[st // 4]
                psv = psS.tile([P, CHUNK], F32, tag="s", name="psv")
                for dt in range(DT):
                    nc.tensor.matmul(
                        psv[:], xc[:, dt, (st % 4) * P:(st % 4 + 1) * P],
                        wv_s[:, dt, ec * CHUNK:(ec + 1) * CHUNK],
                        start=(dt == 0), stop=(dt == DT - 1))
                va_v = va_s.rearrange("p k (h c) -> p k h c", c=DK + 1)
                nh = CHUNK // DK
                nc.vector.tensor_copy(
                    va_v[:, st, ec * nh:(ec + 1) * nh, 0:DK],
                    psv.rearrange("p (h c) -> p h c", c=DK))

            def outproj_tile(qc, qtl, ec):
                psf = psS.tile([P, CHUNK], F32, tag="s", name="psf")
                for j in range(DT):
                    nc.tensor.matmul(
                        psf[:],
                        qy_s[:, j, qc * CHUNK + qtl * P:
                             qc * CHUNK + (qtl + 1) * P],
                        wo_s[:, j, ec * CHUNK:(ec + 1) * CHUNK],
                        start=(j == 0), stop=(j == DT - 1))
                osb = opool.tile([P, CHUNK], BF16, tag="o", name="osb")
                nc.vector.tensor_add(
                    osb[:], psf[:], bob_s[:, ec * CHUNK:(ec + 1) * CHUNK])
                r0 = qc * CHUNK + qtl * P
                nc.sync.dma_start(
                    out[r0:r0 + P, ec * CHUNK:(ec + 1) * CHUNK], osb[:])

            # ---- attention blocks ----
            ex_tiles = {}

            def scores_pair(hp, qc, k, j):
                # both parities of kv block 2k+j -> one tile, concurrent MMs
                kt = 2 * k + j
                qsl = slice(qc * CHUNK, (qc + 1) * CHUNK)
                pst = psS.tile([P, 2, CHUNK], F32, tag="s", name="pst")
                for par in (0, 1):
                    pb = DK * par
                    nc.tensor.matmul(
                        pst[:, par, :],
                        kt_s[pb:pb + DK, hp % KTR, kt * P:(kt + 1) * P],
                        qy_s[pb:pb + DK, hp, qsl],
                        start=True, stop=True)
                t = epool.tile([P, 2, CHUNK], BF16, tag="e", name="ex")
                nc.scalar.activation(t[:], pst[:], Exp)
                ex_tiles[qc, k, j] = t

            psa_open = {}

            def attnv_slot(hp, qc, k):
                for j in (0, 1):
                    kt = 2 * k + j
                    t = ex_tiles.pop((qc, k, j))
                    for par in (0, 1):
                        h = 2 * hp + par
                        key = (qc, par)
                        if kt == 0:
                            psa_open[key] = psA.tile([DK + 1, CHUNK], F32,
                                                     tag="a", name="psa")
                        nc.tensor.matmul(
                            psa_open[key][:],
                            va_s[:, kt, h * (DK + 1):(h + 1) * (DK + 1)],
                            t[:, par, :],
                            start=(kt == 0), stop=(kt == KT_N - 1))

            pending = []

            def flush_normalize():
                for rsb_, par_, hp_, qc_ in pending:
                    psr = psS.tile([DK, CHUNK], F32, tag="s", name="psr")
                    nc.tensor.matmul(psr[:], ones_s[:], rsb_[:],
                                     start=True, stop=True)
                    ysl = qy_s[DK * par_:DK * (par_ + 1), hp_,
                               qc_ * CHUNK:(qc_ + 1) * CHUNK]
                    nc.vector.tensor_mul(ysl, ysl, psr[:])
                pending.clear()

            def normalize(hp, qc):
                for par in (0, 1):
                    psa = psa_open.pop((qc, par))
                    nc.vector.tensor_copy(
                        qy_s[DK * par:DK * (par + 1), hp,
                             qc * CHUNK:(qc + 1) * CHUNK], psa[0:DK, :])
                    rcp = npool.tile([1, CHUNK], F32, tag="rcp", name="rcp")
                    nc.vector.tensor_copy(rcp[:], psa[DK:DK + 1, :])
                    rs = npool.tile([1, CHUNK], F32, tag="rs", name="rs")
                    nc.vector.reciprocal_approx_fast(rs[:], rcp[:])
                    rsb = npool.tile([1, CHUNK], BF16, tag="rsb",
                                     name="rsb", bufs=4)
                    nc.vector.tensor_copy(rsb[:], rs[:])
                    pending.append((rsb, par, hp, qc))

            # ---- ride queue: (deadline_gw, fn); fn emits an even number of
            # "s" allocations. Forced pop 2 windows before deadline, else
            # one entry per window. ----
            rides = []

            def add(dl, fn, earliest=0):
                rides.append((dl, earliest, fn))

            def pair(f1, f2):
                def go():
                    f1()
                    f2()
                return go

            # vproj: st pair needed by attnV(stream0, slot st//2) at window
            # 8 + st//2. Prefetch xv chunk c+1 at st == 4c+2 (after the
            # halves so the ring slot's prior readers are already emitted).
            for st in range(KT_N):
                def go(st=st):
                    vproj_half(st, 0)
                    vproj_half(st, 1)
                    if st % 4 == 2 and st // 4 + 1 < 4:
                        dma_xv(st // 4 + 1)
                add(8 + st // 2, go, 2 + st // 2)
            # kproj pairs for et >= 1 (+ their xk chunk DMAs one pair ahead)
            for et in range(1, ET):
                def go_a(et=et):
                    kproj(et, 0)
                    kproj(et, 1)
                    dma_xk(et, 2)
                    dma_xk(et, 3)

                def go_b(et=et):
                    kproj(et, 2)
                    kproj(et, 3)
                    if et + 1 < ET:
                        dma_xk(et + 1, 0)
                        dma_xk(et + 1, 1)
                # kt ring slot et%KTR: all scores reads of et-KTR must be
                # emitted first -> not before window 16*(et-KTR)+16.
                ek = max(0, 16 * (et - KTR) + 16)
                add(16 * et, go_a, max(ek, 16 * et - 32))
                add(16 * et + 4, go_b, max(ek, 16 * et - 28))
            # qproj pairs for et >= 1
            for et in range(1, ET):
                add(16 * et, pair(lambda et=et: qproj(et, 0),
                                  lambda et=et: qproj(et, 1)),
                    max(0, 16 * et - 24))
            # late wq/wk slices (0 "s" allocs; even = 0)
            for et in range(WR, ET):
                add(16 * et - 8, pair(lambda et=et: dma_wslice(wq_s, wqV, et),
                                      lambda et=et: dma_wslice(wk_s, wkV, et)))
            rides.sort(key=lambda e: e[0])
            ridx = [0]

            def pump(gw):
                popped = 0
                while ridx[0] < len(rides):
                    dl, earliest, fn = rides[ridx[0]]
                    if gw < earliest:
                        break
                    if dl <= gw + 2 or popped == 0:
                        fn()
                        ridx[0] += 1
                        popped += 1
                        if popped >= 4:
                            break
                    else:
                        break

            # ---- lead-in ----
            # sync queue: only JIT-ordered loads (x chunks); bulk weights go
            # on the otherwise-idle vector/gpsimd queues.
            dma_wslice(wq_s, wqV, 0, nc.sync)
            dma_xq(0)
            dma_wslice(wk_s, wkV, 0, nc.sync)
            dma_xk(0, 0)
            dma_xk(0, 1)
            dma_xv(0)
            nc.scalar.dma_start(wv_s[:, :, 0:CHUNK],
                                pdt(wvT)[:, :, 0:CHUNK])
            nc.scalar.dma_start(wv_s[:, :, CHUNK:D],
                                pdt(wvT)[:, :, CHUNK:D])
            nc.scalar.dma_start(
                xq_s[:, 1, :, :], pdt(xqT[:, CHUNK:2 * CHUNK]))
            for et in range(1, WR):
                dma_wslice(wk_s, wkV, et)
                dma_wslice(wq_s, wqV, et)
            nc.gpsimd.dma_start(wo_s[:, 0:4, :], pdt(woT)[:, 0:4, :])
            nc.gpsimd.dma_start(wo_s[:, 4:8, :], pdt(woT)[:, 4:8, :])

            qproj(0, 0)
            kproj(0, 0)

            # remaining et0 work as early rides
            def go_e1():
                kproj(0, 1)
                dma_xk(0, 2)
                dma_xk(0, 3)

            def go_e2():
                kproj(0, 2)
                kproj(0, 3)
                dma_xk(1, 0)
                dma_xk(1, 1)

            def go_e3():
                qproj(0, 1)
            add(2, go_e1)
            add(5, go_e2)
            add(6, go_e3)
            rides.sort(key=lambda e: e[0])

            # ---- main loop: 16 streams x 8 windows ----
            # stream 15 runs attnV(14) at double rate so normalize(14)
            # lands mid-stream and outproj(0) can start before the drain.
            prev14 = [(0, 1), (2, 3), (4, 5), (6,), (7,), (), (), ()]
            for i, (hp, qc) in enumerate(STREAMS):
                for k in range(WIN):
                    gw = 8 * i + k
                    if k == 0 and pending:
                        flush_normalize()
                    if i == 15:
                        for kk in prev14[k]:
                            attnv_slot(*STREAMS[14], kk)
                        if k == 5:
                            normalize(*STREAMS[14])
                        if k == 6:
                            flush_normalize()
                            outproj_tile(0, 0, 0)
                            outproj_tile(0, 0, 1)
                        if k == 7:
                            outproj_tile(0, 1, 0)
                            outproj_tile(0, 1, 1)
                    elif i > 0:
                        attnv_slot(*STREAMS[i - 1], k)
                    scores_pair(hp, qc, k, 0)
                    scores_pair(hp, qc, k, 1)
                    pump(gw)
                if 0 < i < 15:
                    normalize(*STREAMS[i - 1])

            # ---- drain: attnV(7,1) + remaining outproj ----
            for k in range(WIN):
                attnv_slot(7, 1, k)
                if k == 2:
                    outproj_tile(0, 2, 0)
                    outproj_tile(0, 2, 1)
                if k == 4:
                    outproj_tile(0, 3, 0)
                    outproj_tile(0, 3, 1)
            normalize(7, 1)
            flush_normalize()
            for qtl in range(4):
                outproj_tile(1, qtl, 0)
                outproj_tile(1, qtl, 1)

    nc.compile()
    return nc


def _get_nc():
    if "nc" not in _CACHE:
        _CACHE["nc"] = _build_nc()
    return _CACHE["nc"]


def _prep_in_maps(query, key, value, Wq, bq, Wk, bk, Wv, bv, Wo, bo):
    bf16 = ml_dtypes.bfloat16
    query = np.asarray(query, np.float32)
    key = np.asarray(key, np.float32)
    value = np.asarray(value, np.float32)
    Wq, bq = np.asarray(Wq, np.float32), np.asarray(bq, np.float32)
    Wk = np.asarray(Wk, np.float32)
    Wv, bv = np.asarray(Wv, np.float32), np.asarray(bv, np.float32)
    Wo, bo = np.asarray(Wo, np.float32), np.asarray(bo, np.float32)

    def esliced(W):
        # [p, et, dt, m] = W[et*128+m, dt*128+p]
        a = W.reshape(ET, P, DT, P).transpose(3, 0, 2, 1)
        return np.ascontiguousarray(a).reshape(P, ET * DT * P).astype(bf16)

    shared = {
        "wqE": esliced(Wq),
        "wkE": esliced(Wk),
        "wvT": np.ascontiguousarray(Wv.T).astype(bf16),
        "woT": np.ascontiguousarray(Wo.T).astype(bf16),
        "bqs": np.ascontiguousarray((bq / 8.0).reshape(ET, P).T).astype(
            np.float32),
        "bob": np.ascontiguousarray(
            np.broadcast_to(bo + Wo @ bv, (P, D))).astype(bf16),
    }
    xkTs = [np.ascontiguousarray(key[b].T).astype(bf16) for b in range(B)]
    xvTs = [np.ascontiguousarray(value[b].T).astype(bf16) for b in range(B)]

    in_maps = []
    for c in range(NCORES):
        b, half = divmod(c, 2)
        xq = query[b, half * SQ:(half + 1) * SQ, :]
        in_maps.append({
            **shared,
            "xqT": np.ascontiguousarray(xq.T).astype(bf16),
            "xkT": xkTs[b],
            "xvT": xvTs[b],
        })
    return in_maps


def kernel(query, key, value, Wq, bq, Wk, bk, Wv, bv, Wo, bo):
    from concourse.bass_utils import run_bass_kernel_spmd

    nc = _get_nc()
    in_maps = _prep_in_maps(query, key, value, Wq, bq, Wk, bk, Wv, bv,
                            Wo, bo)
    res = run_bass_kernel_spmd(nc, in_maps, list(range(NCORES)))

    outp = np.empty((B, S, D), np.float32)
    for c in range(NCORES):
        b, half = divmod(c, 2)
        outp[b, half * SQ:(half + 1) * SQ, :] = \
            res.results[c]["out"].astype(np.float32)
    return outp
